# revision 40
# baseline (speedup 1.0000x reference)
"""Two-layer GCN (AttributeDecoder) as a distributed Bass kernel on 8 TRN2 NeuronCores.

Math (per reference):
    dis = (deg of A+I)^-1/2
    L1:  relu1 = relu( D @ ((A+I) @ (D @ x)) @ W1 + b1 )   with D = diag(dis)
    L2:  out   = relu( D @ ((A+I) @ (D @ relu1)) @ W2 + b2 )
using (A_hat @ h) @ W == A_hat @ (h @ W) so both layers aggregate 64-wide
features before the dense W matmul.

Sharding: destination nodes (and their in-edges) are partitioned contiguously
across the 8 cores; within a core, nodes are re-assigned to 128-node blocks by
a greedy balance of per-block in-edge counts (per source parity).

Layer 1 messages are raw bf16(x) rows shipped pre-permuted into edge-slot
order (an input-staging gather on the host - the edge normalization is folded
into the host-built selection matrices, so no on-device table build or gather
is needed for L1). Layer 2 messages are gathered on-device with dma_gather
from the AllGathered relu1*dis table (int16 indices address 512B row pairs so
they fit int16; 4 SWDGE queues). SWDGE descriptor generation shares the
GpSimd<->SBUF port, so the steady-state loop avoids DVE entirely: selection
matrices S (S[e, n] = edge-norm weight iff dst(e) == n, dis[dst] folded in)
stream from HBM, aggregation + dense W + rank-1 bias run on TensorE with PSUM
accumulation, and the epilogue is a single Relu on the Activation engine.

The layer-2 table (relu1 * dis) is exchanged with chunked AllGathers that
overlap layer-1 compute; layer-2 gathers read the AllGather output buffer
directly, with slots sorted by source row within each (block, parity) segment
for HBM locality.
"""

import numpy as np
import ml_dtypes

from concourse import bass, mybir, bacc
import concourse.tile as tile
from concourse.bass_utils import run_bass_kernel_spmd

BF16 = ml_dtypes.bfloat16
P = 128
N_CORES = 8
G = 4               # dst blocks per gather group
NQ = 4              # SWDGE queues
CSL = 24            # gather slots per dma_gather call


def _balance_blocks(dE, dO, par_n, nb):
    """Assign nodes to blocks (64 even-id + 64 odd-id slots each) greedily
    minimizing the max per-parity edge load. Returns (block, pos) per node."""
    nsh = len(dE)
    loadE = np.zeros(nb, dtype=np.int64)
    loadO = np.zeros(nb, dtype=np.int64)
    cnt = np.zeros((nb, 2), dtype=np.int64)     # slots used per id-parity
    block = np.zeros(nsh, dtype=np.int64)
    order = np.argsort(-(dE + dO), kind="stable")
    for n in order:
        q = par_n[n]
        cand = np.where(cnt[:, q] < P // 2)[0]
        scores = np.maximum(loadE[cand] + dE[n], loadO[cand] + dO[n])
        b = cand[np.argmin(scores)]
        block[n] = b
        loadE[b] += dE[n]
        loadO[b] += dO[n]
        cnt[b, q] += 1
    # positions: even-id nodes at even positions, odd at odd (keeps the
    # layer-2 table row parity equal to the node id parity)
    pos = np.zeros(nsh, dtype=np.int64)
    ctr = np.zeros((nb, 2), dtype=np.int64)
    for n in range(nsh):
        b, q = block[n], par_n[n]
        pos[n] = 2 * ctr[b, q] + q
        ctr[b, q] += 1
    return block, pos


def _preprocess(x, edge_index, W1, b1, W2, b2):
    n = x.shape[0]
    f1 = x.shape[1]
    f2 = W2.shape[1]
    assert n % N_CORES == 0
    nsh = n // N_CORES
    nb = (nsh + P - 1) // P
    nsh_pad = nb * P
    assert nsh % 2 == 0

    ei = np.asarray(edge_index).astype(np.int64)
    src = ei[0].copy()
    dst = ei[1].copy()
    x_bf = np.asarray(x, dtype=np.float32).astype(BF16)

    deg = np.bincount(dst, minlength=n).astype(np.float32) + 1.0  # + self loop
    dis = (1.0 / np.sqrt(deg)).astype(np.float32)

    owner = dst // nsh
    par = (src % 2).astype(np.int64)

    # chunked AllGather: small first chunk starts the exchange early, small
    # last chunk keeps the final-chunk exposure (which gates all of L2) low
    if nb >= 16:
        nch = 5
        cb = [0, min(G, nb), nb // 3, 2 * nb // 3, nb - max(1, nb // 7), nb]
    else:
        nch = 4
        cb = [0, min(G, nb)] if nb > G else [0, nb]
        while len(cb) < nch + 1:
            cb.append(min(nb, cb[-1] + max(1, (nb - cb[-1]) // (nch + 1 - len(cb)))))
        cb[-1] = nb
    csz = [(cb[k + 1] - cb[k]) * P for k in range(nch)]  # rows per core/chunk
    off = np.zeros(nch + 1, dtype=np.int64)
    for k in range(nch):
        off[k + 1] = off[k] + N_CORES * csz[k]

    # per-core balanced node->(block, pos) assignment
    blocks_all = np.zeros(n, dtype=np.int64)
    pos_all = np.zeros(n, dtype=np.int64)
    for c in range(N_CORES):
        lo, hi = c * nsh, (c + 1) * nsh
        m = (dst >= lo) & (dst < hi)
        dloc = dst[m] - lo
        dE = np.bincount(dloc[par[m] == 0], minlength=nsh)
        dO = np.bincount(dloc[par[m] == 1], minlength=nsh)
        par_n = np.arange(nsh) % 2
        blk, pos = _balance_blocks(dE, dO, par_n, nb)
        blocks_all[lo:hi] = blk
        pos_all[lo:hi] = pos

    # layer-2 table row for each global node (chunk-major AllGather layout)
    cb_a = np.asarray(cb)
    csz_a = np.asarray(csz)
    chunk_of = np.searchsorted(cb_a, blocks_all, side="right") - 1
    row2_all = (
        off[chunk_of]
        + (np.arange(n) // nsh) * csz_a[chunk_of]
        + (blocks_all - cb_a[chunk_of]) * P
        + pos_all
    )
    # row parity must equal node-id parity (for the shared parity split)
    assert ((row2_all % 2) == (np.arange(n) % 2)).all()

    # per-(core, block, parity) counts -> uniform external subtile count T2
    e_blk = blocks_all[dst]
    cnt = np.zeros((N_CORES, nb, 2), dtype=np.int64)
    np.add.at(cnt, (owner, e_blk, par), 1)
    T2 = max(1, int((cnt.max() + P - 1) // P))
    TS = 2 * T2                       # external subtile slots per block
    TT = TS + 1                       # + self subtile

    n_groups = (nb + G - 1) // G

    def wrap16(flat):
        cols = len(flat) // 16
        img = flat.reshape(cols, 16).T
        return np.tile(img, (8, 1)).astype(np.int16)

    def call_order(a):
        """[nb, TS, ...] -> call-order slot stream [(slots), ...]"""
        segs = []
        for g in range(n_groups):
            g0, g1 = g * G, min(g * G + G, nb)
            segs.append(a[g0:g1, :T2].reshape((-1, P) + a.shape[3:]))
            segs.append(a[g0:g1, T2:].reshape((-1, P) + a.shape[3:]))
        return np.concatenate(segs)

    in_maps = []
    call_bounds = []
    for c in range(N_CORES):
        lo = c * nsh
        m = owner == c
        s_c = src[m]
        b_c = e_blk[m]
        p_c = pos_all[dst[m]]
        g_c = b_c * 2 + par[m]
        cnt_c = cnt[c].reshape(-1)
        start = np.zeros(nb * 2, dtype=np.int64)
        start[1:] = np.cumsum(cnt_c)[:-1]

        # own-node dis per (block, pos), zero at empty slots
        node_at = np.full(nsh_pad, -1, dtype=np.int64)
        node_at[blocks_all[lo : lo + nsh] * P + pos_all[lo : lo + nsh]] = (
            np.arange(nsh)
        )
        occ = node_at >= 0
        dv = np.zeros(nsh_pad, dtype=np.float32)
        dv[occ] = dis[lo + node_at[occ]]
        dv2d = dv.reshape(nb, P)

        def layer_order(sort_key):
            order = np.lexsort((sort_key, g_c))
            g_o = g_c[order]
            slot = np.arange(len(g_o)) - start[g_o]
            return order, g_o, slot

        def build_S(order, g_o, slot, edge_w, self_w):
            """S[b, t, e, n] = per-edge weight folded with dst selection."""
            S = np.zeros((nb, TT, P, P), dtype=np.float32)
            b_o = b_c[order]
            t_abs = (g_o % 2) * T2 + slot // P
            lane = slot % P
            p_o = p_c[order]
            S.reshape(-1, P)[(b_o * TT + t_abs) * P + lane, p_o] = edge_w
            ar = np.arange(P)
            S[:, TS, ar, ar] = self_w[:, ar]
            return np.ascontiguousarray(
                S.transpose(2, 0, 1, 3).reshape(P, nb * TT * P)
            ).astype(BF16)

        # ---- L1: messages are raw bf16(x) shipped in slot order;
        # S1 folds dis[dst]^2 * dis[src] (self: dis^3) ----
        order1, g_o1, slot1 = layer_order(s_c)
        M = np.zeros((nb, TS, P, f1), dtype=BF16)
        t1 = (g_o1 % 2) * T2 + slot1 // P
        M[b_c[order1], t1, slot1 % P] = x_bf[s_c[order1]]
        m1_img = np.ascontiguousarray(
            call_order(M).transpose(1, 0, 2).reshape(P, -1)
        )
        ew1 = dv2d[b_c[order1], p_c[order1]] ** 2 * dis[s_c[order1]]
        s1_img = build_S(order1, g_o1, slot1, ew1, dv2d ** 3)

        # ---- L2: gathered from AllGathered table (rows already carry
        # dis[src]); S2 folds dis[dst] (self: dis). Slots are sorted by
        # source row within each (block, parity) segment, and gather calls
        # are issued subtile-layer-major so early calls touch only low table
        # rows: a tight per-call in_ap row bound then lets them start as
        # soon as the covering AllGather chunk has landed. ----
        r2 = row2_all[s_c]
        order2, g_o2, slot2 = layer_order(r2 >> 1)
        srch = np.zeros(nb * TS * P, dtype=np.int64)
        srch[g_o2 * (T2 * P) + slot2] = r2[order2] >> 1
        grid = srch.reshape(nb, TS, P)
        stream = []
        for g in range(n_groups):
            g0, g1 = g * G, min(g * G + G, nb)
            for tsub in range(T2):
                for pr in (0, 1):
                    for b in range(g0, g1):
                        stream.append(grid[b, pr * T2 + tsub])
        stream = np.stack(stream)                      # [cols, P]
        src2_img = wrap16(stream.reshape(-1))
        ew2 = dv2d[b_c[order2], p_c[order2]]
        s2_img = build_S(order2, g_o2, slot2, ew2, dv2d)

        xo = np.zeros((nsh_pad, f1), dtype=BF16)
        xo[occ] = x_bf[lo + node_at[occ]]

        # per-gather-call max pair index (exclusive bound), issue order
        bounds_c = []
        sbase = 0
        for g in range(n_groups):
            gb = min(g * G + G, nb) - g * G
            ncols = 2 * gb * T2
            for s0 in range(0, ncols, CSL):
                s1_ = min(s0 + CSL, ncols)
                bounds_c.append(
                    int(stream[sbase + s0 : sbase + s1_].max()) + 1
                )
            sbase += ncols
        call_bounds.append(bounds_c)

        in_maps.append(
            {"src2": src2_img, "m1": m1_img, "s1": s1_img, "s2": s2_img,
             "d1r": dv[None, :].astype(BF16), "xo": xo, "node_at": node_at}
        )

    shared = {
        "w1": np.asarray(W1, dtype=np.float32).astype(BF16),
        "w2": np.asarray(W2, dtype=np.float32).astype(BF16),
        "b1r": np.asarray(b1, dtype=np.float32)[None, :].astype(BF16),
        "b2r": np.asarray(b2, dtype=np.float32)[None, :].astype(BF16),
    }
    for mp in in_maps:
        mp.update(shared)

    # SPMD: one program for all cores -> per-call bound = max over cores
    bounds = [max(bc[i] for bc in call_bounds)
              for i in range(len(call_bounds[0]))]
    cfg = dict(n=n, f1=f1, f2=f2, nsh=nsh, nb=nb, nsh_pad=nsh_pad, T2=T2,
               TS=TS, TT=TT, n_groups=n_groups,
               nch=nch, cb=cb, csz=csz, off=off.tolist(), bounds=bounds)
    return in_maps, cfg


def _pairs_ap(handle, n_pairs, f1):
    """view table rows as items of row PAIRS: item k -> rows (2k, 2k+1)"""
    ap = handle.ap()
    return bass.AP(ap.tensor, 0, [[2 * f1, n_pairs], [1, 2 * f1]])


def _build(cfg):
    nb, T2, TS, TT = (cfg[k] for k in ("nb", "T2", "TS", "TT"))
    bounds = cfg["bounds"]
    f1, f2, nsh_pad, n_groups = (
        cfg[k] for k in ("f1", "f2", "nsh_pad", "n_groups"))
    nch, cb, csz, off = (cfg[k] for k in ("nch", "cb", "csz", "off"))
    dt = mybir.dt
    idx_cols = nb * TS * P // 16

    nc = bacc.Bacc("TRN2", target_bir_lowering=False, debug=False,
                   num_devices=N_CORES, num_swdge_queues=NQ)

    xo = nc.dram_tensor("xo", [nsh_pad, f1], dt.bfloat16, kind="ExternalInput")
    w1 = nc.dram_tensor("w1", [f1, f1], dt.bfloat16, kind="ExternalInput")
    w2 = nc.dram_tensor("w2", [f1, f2], dt.bfloat16, kind="ExternalInput")
    b1r = nc.dram_tensor("b1r", [1, f1], dt.bfloat16, kind="ExternalInput")
    b2r = nc.dram_tensor("b2r", [1, f2], dt.bfloat16, kind="ExternalInput")
    d1r = nc.dram_tensor("d1r", [1, nsh_pad], dt.bfloat16, kind="ExternalInput")
    src2 = nc.dram_tensor("src2", [P, idx_cols], dt.int16, kind="ExternalInput")
    m1 = nc.dram_tensor("m1", [P, nb * TS * f1], dt.bfloat16,
                        kind="ExternalInput")
    s1 = nc.dram_tensor("s1", [P, nb * TT * P], dt.bfloat16,
                        kind="ExternalInput")
    s2 = nc.dram_tensor("s2", [P, nb * TT * P], dt.bfloat16,
                        kind="ExternalInput")
    out = nc.dram_tensor("out", [nsh_pad, f2], dt.float32, kind="ExternalOutput")

    r1s_own = nc.dram_tensor("r1s_own", [nsh_pad, f1], dt.bfloat16)
    r1s_full = nc.dram_tensor("r1s_full", [N_CORES * nsh_pad, f1], dt.bfloat16,
                              addr_space="Shared")
    cc_warm_in = nc.dram_tensor("cc_warm_in", [1, P], dt.float32)
    cc_warm_out = nc.dram_tensor("cc_warm_out", [N_CORES, P], dt.float32,
                                 addr_space="Shared")

    with tile.TileContext(nc) as tc:
        with (
            tc.tile_pool(name="const", bufs=1) as constp,
            tc.tile_pool(name="msg", bufs=4) as msgp,
            tc.tile_pool(name="m1p", bufs=1) as m1p,
            tc.tile_pool(name="smat", bufs=3) as smatp,
            tc.tile_pool(name="eplg", bufs=3) as eplgp,
            tc.tile_pool(name="acc", bufs=1) as accp,
            tc.tile_pool(name="ps1", bufs=2, space="PSUM") as ps1p,
            tc.tile_pool(name="ps2", bufs=2, space="PSUM") as ps2p,
        ):
            # warm up the collectives firmware under the prologue
            nc.gpsimd.collective_compute(
                "AllGather",
                mybir.AluOpType.bypass,
                replica_groups=[list(range(N_CORES))],
                ins=[cc_warm_in.ap().opt()],
                outs=[cc_warm_out.ap().opt()],
            )
            # ---- constants ----
            w1_sb = constp.tile([f1, f1], dt.bfloat16)
            nc.sync.dma_start(out=w1_sb[:], in_=w1.ap())
            w2_sb = constp.tile([f1, f2], dt.bfloat16)
            nc.sync.dma_start(out=w2_sb[:], in_=w2.ap())
            b1r_sb = constp.tile([1, f1], dt.bfloat16)
            nc.sync.dma_start(out=b1r_sb[:], in_=b1r.ap())
            b2r_sb = constp.tile([1, f2], dt.bfloat16)
            nc.sync.dma_start(out=b2r_sb[:], in_=b2r.ap())
            d1r_sb = constp.tile([1, nsh_pad], dt.bfloat16)
            nc.sync.dma_start(out=d1r_sb[:], in_=d1r.ap())
            ones_sb = constp.tile([1, P], dt.bfloat16)
            nc.vector.memset(ones_sb[:], 1.0)
            xo_sb = constp.tile([P, nb, f1], dt.bfloat16)
            nc.sync.dma_start(out=xo_sb[:],
                              in_=xo.ap().rearrange("(b p) f -> p b f", p=P))
            src2_sb = constp.tile([P, idx_cols], dt.int16)
            nc.scalar.dma_start(out=src2_sb[:], in_=src2.ap())

            qctr = [0]

            def layer(gather, src_sb, m_dram, s_dram, selftab, w_sb,
                      bias_lhsT, b_sb, fo, emit):
                tab = gather
                callno = [0]
                slot_base = 0
                for g in range(n_groups):
                    g0, g1 = g * G, min(g * G + G, nb)
                    gb = g1 - g0
                    half = gb * T2
                    s_t = smatp.tile([P, G * TT, P], dt.bfloat16, tag="smat")
                    # alternate HWDGE queues so the big S loads don't
                    # serialize behind each other on one sequencer
                    s_eng = nc.sync if g % 2 == 0 else nc.scalar
                    s_eng.dma_start(
                        out=s_t[:, : gb * TT, :],
                        in_=s_dram.ap()[:, g0 * TT * P : g1 * TT * P],
                    )
                    if gather:
                        msg = msgp.tile([P, G * TS, 2 * f1], dt.bfloat16,
                                        tag="msg")
                        for s0 in range(0, 2 * half, CSL):
                            s1_ = min(s0 + CSL, 2 * half)
                            i0 = (slot_base + s0) * P
                            n_idx = (s1_ - s0) * P
                            nc.gpsimd.dma_gather(
                                out_ap=msg[:, s0:s1_, :],
                                in_ap=_pairs_ap(tab, bounds[callno[0]], f1),
                                idxs_ap=src_sb[:, i0 // 16 : (i0 + n_idx) // 16],
                                num_idxs=n_idx,
                                num_idxs_reg=n_idx,
                                elem_size=2 * f1,
                                elem_step=2 * f1,
                                single_packet=False,
                                queue_num=qctr[0] % NQ,
                            )
                            qctr[0] += 1
                            callno[0] += 1
                    else:
                        msg = m1p.tile([P, G * TS, f1], dt.bfloat16, tag="m1")
                        m_eng = nc.scalar if g % 2 == 0 else nc.sync
                        m_eng.dma_start(
                            out=msg[:, : 2 * half, :],
                            in_=m_dram.ap()[
                                :, slot_base * f1 : (slot_base + 2 * half) * f1
                            ],
                        )
                    for j, b in enumerate(range(g0, g1)):
                        ps1 = ps1p.tile([f1, P], dt.float32, space="PSUM",
                                        tag="ps1")
                        for t in range(TT):
                            if t < TS:
                                parity, tsub = (0, t) if t < T2 else (1, t - T2)
                                if gather:
                                    # stream order: subtile-layer-major
                                    col = (tsub * 2 + parity) * gb + j
                                    lhsT = msg[:, col,
                                               parity * f1 : parity * f1 + f1]
                                else:
                                    lhsT = msg[:, parity * half + j * T2 + tsub,
                                               :]
                            else:
                                lhsT = selftab[:, b, :f1]
                            nc.tensor.matmul(
                                out=ps1[:],
                                lhsT=lhsT,
                                rhs=s_t[:, j * TT + t, :],
                                start=(t == 0),
                                stop=(t == TT - 1),
                            )
                        aggT = eplgp.tile([f1, P], dt.bfloat16, tag="aggT")
                        nc.scalar.copy(aggT[:], ps1[:])
                        ps2 = ps2p.tile([P, fo], dt.float32, space="PSUM",
                                        tag="ps2")
                        nc.tensor.matmul(out=ps2[:], lhsT=aggT[:], rhs=w_sb[:],
                                         start=True, stop=False)
                        nc.tensor.matmul(out=ps2[:], lhsT=bias_lhsT(b),
                                         rhs=b_sb[:], start=False, stop=True)
                        emit(b, ps2)
                    slot_base += gb * TS

            # ---- L1 ----
            r1s_sb = accp.tile([P, nb, f1], dt.bfloat16)
            r1s_own_r = r1s_own.ap().rearrange("(b p) f -> p b f", p=P)
            next_chunk = [0]

            def emit1(b, ps2):
                nc.scalar.activation(
                    out=r1s_sb[:, b, :], in_=ps2[:],
                    func=mybir.ActivationFunctionType.Relu,
                )
                k = next_chunk[0]
                if k < nch and b == cb[k + 1] - 1:
                    nc.sync.dma_start(out=r1s_own_r[:, cb[k] : cb[k + 1], :],
                                      in_=r1s_sb[:, cb[k] : cb[k + 1], :])
                    nc.gpsimd.collective_compute(
                        "AllGather",
                        mybir.AluOpType.bypass,
                        replica_groups=[list(range(N_CORES))],
                        ins=[r1s_own.ap()[cb[k] * P : cb[k + 1] * P, :].opt()],
                        outs=[r1s_full.ap()[off[k] : off[k + 1], :].opt()],
                    )
                    next_chunk[0] += 1

            layer(None, None, m1, s1, xo_sb, w1_sb,
                  lambda b: d1r_sb[0:1, b * P : (b + 1) * P], b1r_sb, f1, emit1)

            # ---- L2 ----
            out_r = out.ap().rearrange("(b p) f -> p b f", p=P)

            def emit2(b, ps2):
                ot = eplgp.tile([P, f2], dt.float32, tag="ot")
                nc.scalar.activation(
                    out=ot[:], in_=ps2[:],
                    func=mybir.ActivationFunctionType.Relu,
                )
                o_eng = nc.sync if b % 2 == 0 else nc.scalar
                o_eng.dma_start(out=out_r[:, b, :], in_=ot[:])

            layer(r1s_full, src2_sb, None, s2, r1s_sb,
                  w2_sb, lambda b: ones_sb[0:1, :], b2r_sb, f2, emit2)

    nc.compile()
    return nc


_CACHE = {}


def kernel(x, edge_index, W1, b1, W2, b2, _want_profile=False):
    x = np.asarray(x)
    in_maps, cfg = _preprocess(x, edge_index, W1, b1, W2, b2)
    key = (cfg["n"], cfg["f1"], cfg["f2"], cfg["T2"], tuple(cfg["bounds"]))
    if key not in _CACHE:
        _CACHE[key] = _build(cfg)
    nc = _CACHE[key]
    node_ats = [m.pop("node_at") for m in in_maps]
    res = run_bass_kernel_spmd(
        nc, in_maps, core_ids=list(range(N_CORES)), trace=_want_profile
    )
    nsh = cfg["nsh"]
    full = np.empty((cfg["n"], cfg["f2"]), dtype=np.float32)
    for c in range(N_CORES):
        o = res.results[c]["out"]
        na = node_ats[c]
        occ = na >= 0
        full[c * nsh + na[occ]] = o[occ]
    if _want_profile:
        return full, res
    return full


# revision 42
# speedup vs baseline: 1.0106x; 1.0106x over previous
"""Two-layer GCN (AttributeDecoder) as a distributed Bass kernel on 8 TRN2 NeuronCores.

Math (per reference):
    dis = (deg of A+I)^-1/2
    L1:  relu1 = relu( D @ ((A+I) @ (D @ x)) @ W1 + b1 )   with D = diag(dis)
    L2:  out   = relu( D @ ((A+I) @ (D @ relu1)) @ W2 + b2 )
using (A_hat @ h) @ W == A_hat @ (h @ W) so both layers aggregate 64-wide
features before the dense W matmul.

Sharding: destination nodes (and their in-edges) are partitioned contiguously
across the 8 cores; within a core, nodes are re-assigned to 128-node blocks by
a greedy balance of per-block in-edge counts (per source parity).

Layer 1 messages are raw bf16(x) rows shipped pre-permuted into edge-slot
order (an input-staging gather on the host - the edge normalization is folded
into the host-built selection matrices, so no on-device table build or gather
is needed for L1). Layer 2 messages are gathered on-device with dma_gather
from the AllGathered relu1*dis table (int16 indices address 512B row pairs so
they fit int16; 4 SWDGE queues). SWDGE descriptor generation shares the
GpSimd<->SBUF port, so the steady-state loop avoids DVE entirely: selection
matrices S (S[e, n] = edge-norm weight iff dst(e) == n, dis[dst] folded in)
stream from HBM, aggregation + dense W + rank-1 bias run on TensorE with PSUM
accumulation, and the epilogue is a single Relu on the Activation engine.

The layer-2 table (relu1 * dis) is exchanged with chunked AllGathers that
overlap layer-1 compute; layer-2 gathers read the AllGather output buffer
directly, with slots sorted by source row within each (block, parity) segment
for HBM locality.
"""

import numpy as np
import ml_dtypes

from concourse import bass, mybir, bacc
import concourse.tile as tile
from concourse.bass_utils import run_bass_kernel_spmd

BF16 = ml_dtypes.bfloat16
P = 128
N_CORES = 8
G = 4               # dst blocks per gather group
NQ = 4              # SWDGE queues
CSL = 24            # gather slots per dma_gather call


def _balance_blocks(dE, dO, par_n, nb):
    """Assign nodes to blocks (64 even-id + 64 odd-id slots each) greedily
    minimizing the max per-parity edge load. Returns (block, pos) per node."""
    nsh = len(dE)
    loadE = np.zeros(nb, dtype=np.int64)
    loadO = np.zeros(nb, dtype=np.int64)
    cnt = np.zeros((nb, 2), dtype=np.int64)     # slots used per id-parity
    block = np.zeros(nsh, dtype=np.int64)
    order = np.argsort(-(dE + dO), kind="stable")
    for n in order:
        q = par_n[n]
        cand = np.where(cnt[:, q] < P // 2)[0]
        scores = np.maximum(loadE[cand] + dE[n], loadO[cand] + dO[n])
        b = cand[np.argmin(scores)]
        block[n] = b
        loadE[b] += dE[n]
        loadO[b] += dO[n]
        cnt[b, q] += 1
    # positions: even-id nodes at even positions, odd at odd (keeps the
    # layer-2 table row parity equal to the node id parity)
    pos = np.zeros(nsh, dtype=np.int64)
    ctr = np.zeros((nb, 2), dtype=np.int64)
    for n in range(nsh):
        b, q = block[n], par_n[n]
        pos[n] = 2 * ctr[b, q] + q
        ctr[b, q] += 1
    return block, pos


def _preprocess(x, edge_index, W1, b1, W2, b2):
    n = x.shape[0]
    f1 = x.shape[1]
    f2 = W2.shape[1]
    assert n % N_CORES == 0
    nsh = n // N_CORES
    nb = (nsh + P - 1) // P
    nsh_pad = nb * P
    assert nsh % 2 == 0

    ei = np.asarray(edge_index).astype(np.int64)
    src = ei[0].copy()
    dst = ei[1].copy()
    x_bf = np.asarray(x, dtype=np.float32).astype(BF16)

    deg = np.bincount(dst, minlength=n).astype(np.float32) + 1.0  # + self loop
    dis = (1.0 / np.sqrt(deg)).astype(np.float32)

    owner = dst // nsh
    par = (src % 2).astype(np.int64)

    # chunked AllGather: small first chunk starts the exchange early, small
    # last chunk keeps the final-chunk exposure (which gates all of L2) low
    if nb >= 16:
        nch = 5
        cb = [0, min(G, nb), nb // 3, 2 * nb // 3, nb - max(1, nb // 7), nb]
    else:
        nch = 4
        cb = [0, min(G, nb)] if nb > G else [0, nb]
        while len(cb) < nch + 1:
            cb.append(min(nb, cb[-1] + max(1, (nb - cb[-1]) // (nch + 1 - len(cb)))))
        cb[-1] = nb
    csz = [(cb[k + 1] - cb[k]) * P for k in range(nch)]  # rows per core/chunk
    off = np.zeros(nch + 1, dtype=np.int64)
    for k in range(nch):
        off[k + 1] = off[k] + N_CORES * csz[k]

    # per-core balanced node->(block, pos) assignment
    blocks_all = np.zeros(n, dtype=np.int64)
    pos_all = np.zeros(n, dtype=np.int64)
    for c in range(N_CORES):
        lo, hi = c * nsh, (c + 1) * nsh
        m = (dst >= lo) & (dst < hi)
        dloc = dst[m] - lo
        dE = np.bincount(dloc[par[m] == 0], minlength=nsh)
        dO = np.bincount(dloc[par[m] == 1], minlength=nsh)
        par_n = np.arange(nsh) % 2
        blk, pos = _balance_blocks(dE, dO, par_n, nb)
        blocks_all[lo:hi] = blk
        pos_all[lo:hi] = pos

    # layer-2 table row for each global node (chunk-major AllGather layout)
    cb_a = np.asarray(cb)
    csz_a = np.asarray(csz)
    chunk_of = np.searchsorted(cb_a, blocks_all, side="right") - 1
    row2_all = (
        off[chunk_of]
        + (np.arange(n) // nsh) * csz_a[chunk_of]
        + (blocks_all - cb_a[chunk_of]) * P
        + pos_all
    )
    # row parity must equal node-id parity (for the shared parity split)
    assert ((row2_all % 2) == (np.arange(n) % 2)).all()

    # per-(core, block, parity) counts -> uniform external subtile count T2
    e_blk = blocks_all[dst]
    cnt = np.zeros((N_CORES, nb, 2), dtype=np.int64)
    np.add.at(cnt, (owner, e_blk, par), 1)
    T2 = max(1, int((cnt.max() + P - 1) // P))
    TS = 2 * T2                       # external subtile slots per block
    TT = TS + 1                       # + self subtile

    n_groups = (nb + G - 1) // G

    def wrap16(flat):
        cols = len(flat) // 16
        img = flat.reshape(cols, 16).T
        return np.tile(img, (8, 1)).astype(np.int16)

    def call_order(a):
        """[nb, TS, ...] -> call-order slot stream [(slots), ...]"""
        segs = []
        for g in range(n_groups):
            g0, g1 = g * G, min(g * G + G, nb)
            segs.append(a[g0:g1, :T2].reshape((-1, P) + a.shape[3:]))
            segs.append(a[g0:g1, T2:].reshape((-1, P) + a.shape[3:]))
        return np.concatenate(segs)

    in_maps = []
    call_bounds = []
    for c in range(N_CORES):
        lo = c * nsh
        m = owner == c
        s_c = src[m]
        b_c = e_blk[m]
        p_c = pos_all[dst[m]]
        g_c = b_c * 2 + par[m]
        cnt_c = cnt[c].reshape(-1)
        start = np.zeros(nb * 2, dtype=np.int64)
        start[1:] = np.cumsum(cnt_c)[:-1]

        # own-node dis per (block, pos), zero at empty slots
        node_at = np.full(nsh_pad, -1, dtype=np.int64)
        node_at[blocks_all[lo : lo + nsh] * P + pos_all[lo : lo + nsh]] = (
            np.arange(nsh)
        )
        occ = node_at >= 0
        dv = np.zeros(nsh_pad, dtype=np.float32)
        dv[occ] = dis[lo + node_at[occ]]
        dv2d = dv.reshape(nb, P)

        def layer_order(sort_key):
            order = np.lexsort((sort_key, g_c))
            g_o = g_c[order]
            slot = np.arange(len(g_o)) - start[g_o]
            return order, g_o, slot

        def build_S(order, g_o, slot, edge_w, self_w):
            """S[b, t, e, n] = per-edge weight folded with dst selection."""
            S = np.zeros((nb, TT, P, P), dtype=np.float32)
            b_o = b_c[order]
            t_abs = (g_o % 2) * T2 + slot // P
            lane = slot % P
            p_o = p_c[order]
            S.reshape(-1, P)[(b_o * TT + t_abs) * P + lane, p_o] = edge_w
            ar = np.arange(P)
            S[:, TS, ar, ar] = self_w[:, ar]
            return np.ascontiguousarray(
                S.transpose(2, 0, 1, 3).reshape(P, nb * TT * P)
            ).astype(BF16)

        # ---- L1: messages are raw bf16(x) shipped in slot order;
        # S1 folds dis[dst]^2 * dis[src] (self: dis^3) ----
        order1, g_o1, slot1 = layer_order(s_c)
        M = np.zeros((nb, TS, P, f1), dtype=BF16)
        t1 = (g_o1 % 2) * T2 + slot1 // P
        M[b_c[order1], t1, slot1 % P] = x_bf[s_c[order1]]
        m1_img = np.ascontiguousarray(
            call_order(M).transpose(1, 0, 2).reshape(P, -1)
        )
        ew1 = dv2d[b_c[order1], p_c[order1]] ** 2 * dis[s_c[order1]]
        s1_img = build_S(order1, g_o1, slot1, ew1, dv2d ** 3)

        # ---- L2: gathered from AllGathered table (rows already carry
        # dis[src]); S2 folds dis[dst] (self: dis). Slots are sorted by
        # source row within each (block, parity) segment, and gather calls
        # are issued subtile-layer-major so early calls touch only low table
        # rows: a tight per-call in_ap row bound then lets them start as
        # soon as the covering AllGather chunk has landed. ----
        r2 = row2_all[s_c]
        order2, g_o2, slot2 = layer_order(r2 >> 1)
        srch = np.zeros(nb * TS * P, dtype=np.int64)
        srch[g_o2 * (T2 * P) + slot2] = r2[order2] >> 1
        grid = srch.reshape(nb, TS, P)
        stream = []
        for g in range(n_groups):
            g0, g1 = g * G, min(g * G + G, nb)
            for tsub in range(T2):
                for pr in (0, 1):
                    for b in range(g0, g1):
                        stream.append(grid[b, pr * T2 + tsub])
        stream = np.stack(stream)                      # [cols, P]
        src2_img = wrap16(stream.reshape(-1))
        ew2 = dv2d[b_c[order2], p_c[order2]]
        s2_img = build_S(order2, g_o2, slot2, ew2, dv2d)

        xo = np.zeros((nsh_pad, f1), dtype=BF16)
        xo[occ] = x_bf[lo + node_at[occ]]

        # per-gather-call max pair index (exclusive bound), issue order
        bounds_c = []
        sbase = 0
        for g in range(n_groups):
            gb = min(g * G + G, nb) - g * G
            ncols = 2 * gb * T2
            for s0 in range(0, ncols, CSL):
                s1_ = min(s0 + CSL, ncols)
                bounds_c.append(
                    int(stream[sbase + s0 : sbase + s1_].max()) + 1
                )
            sbase += ncols
        call_bounds.append(bounds_c)

        in_maps.append(
            {"src2": src2_img, "m1": m1_img, "s1": s1_img, "s2": s2_img,
             "d1r": dv[None, :].astype(BF16), "xo": xo, "node_at": node_at}
        )

    shared = {
        "w1": np.asarray(W1, dtype=np.float32).astype(BF16),
        "w2": np.asarray(W2, dtype=np.float32).astype(BF16),
        "b1r": np.asarray(b1, dtype=np.float32)[None, :].astype(BF16),
        "b2r": np.asarray(b2, dtype=np.float32)[None, :].astype(BF16),
    }
    for mp in in_maps:
        mp.update(shared)

    # SPMD: one program for all cores -> per-call bound = max over cores
    bounds = [max(bc[i] for bc in call_bounds)
              for i in range(len(call_bounds[0]))]
    cfg = dict(n=n, f1=f1, f2=f2, nsh=nsh, nb=nb, nsh_pad=nsh_pad, T2=T2,
               TS=TS, TT=TT, n_groups=n_groups,
               nch=nch, cb=cb, csz=csz, off=off.tolist(), bounds=bounds)
    return in_maps, cfg


def _pairs_ap(handle, n_pairs, f1):
    """view table rows as items of row PAIRS: item k -> rows (2k, 2k+1)"""
    ap = handle.ap()
    return bass.AP(ap.tensor, 0, [[2 * f1, n_pairs], [1, 2 * f1]])


def _build(cfg):
    nb, T2, TS, TT = (cfg[k] for k in ("nb", "T2", "TS", "TT"))
    bounds = cfg["bounds"]
    f1, f2, nsh_pad, n_groups = (
        cfg[k] for k in ("f1", "f2", "nsh_pad", "n_groups"))
    nch, cb, csz, off = (cfg[k] for k in ("nch", "cb", "csz", "off"))
    dt = mybir.dt
    idx_cols = nb * TS * P // 16

    nc = bacc.Bacc("TRN2", target_bir_lowering=False, debug=False,
                   num_devices=N_CORES, num_swdge_queues=NQ)

    xo = nc.dram_tensor("xo", [nsh_pad, f1], dt.bfloat16, kind="ExternalInput")
    w1 = nc.dram_tensor("w1", [f1, f1], dt.bfloat16, kind="ExternalInput")
    w2 = nc.dram_tensor("w2", [f1, f2], dt.bfloat16, kind="ExternalInput")
    b1r = nc.dram_tensor("b1r", [1, f1], dt.bfloat16, kind="ExternalInput")
    b2r = nc.dram_tensor("b2r", [1, f2], dt.bfloat16, kind="ExternalInput")
    d1r = nc.dram_tensor("d1r", [1, nsh_pad], dt.bfloat16, kind="ExternalInput")
    src2 = nc.dram_tensor("src2", [P, idx_cols], dt.int16, kind="ExternalInput")
    m1 = nc.dram_tensor("m1", [P, nb * TS * f1], dt.bfloat16,
                        kind="ExternalInput")
    s1 = nc.dram_tensor("s1", [P, nb * TT * P], dt.bfloat16,
                        kind="ExternalInput")
    s2 = nc.dram_tensor("s2", [P, nb * TT * P], dt.bfloat16,
                        kind="ExternalInput")
    out = nc.dram_tensor("out", [nsh_pad, f2], dt.float32, kind="ExternalOutput")

    r1s_own = nc.dram_tensor("r1s_own", [nsh_pad, f1], dt.bfloat16)
    r1s_full = nc.dram_tensor("r1s_full", [N_CORES * nsh_pad, f1], dt.bfloat16,
                              addr_space="Shared")
    cc_warm_in = nc.dram_tensor("cc_warm_in", [1, P], dt.float32)
    cc_warm_out = nc.dram_tensor("cc_warm_out", [N_CORES, P], dt.float32,
                                 addr_space="Shared")

    with tile.TileContext(nc) as tc:
        with (
            tc.tile_pool(name="const", bufs=1) as constp,
            tc.tile_pool(name="msg", bufs=3) as msgp,
            tc.tile_pool(name="m1p", bufs=1) as m1p,
            tc.tile_pool(name="smat", bufs=3) as smatp,
            tc.tile_pool(name="eplg", bufs=3) as eplgp,
            tc.tile_pool(name="acc", bufs=1) as accp,
            tc.tile_pool(name="ps1", bufs=2, space="PSUM") as ps1p,
            tc.tile_pool(name="ps2", bufs=2, space="PSUM") as ps2p,
        ):
            # warm up the collectives firmware under the prologue
            nc.gpsimd.collective_compute(
                "AllGather",
                mybir.AluOpType.bypass,
                replica_groups=[list(range(N_CORES))],
                ins=[cc_warm_in.ap().opt()],
                outs=[cc_warm_out.ap().opt()],
            )
            # ---- constants ----
            w1_sb = constp.tile([f1, f1], dt.bfloat16)
            nc.sync.dma_start(out=w1_sb[:], in_=w1.ap())
            w2_sb = constp.tile([f1, f2], dt.bfloat16)
            nc.sync.dma_start(out=w2_sb[:], in_=w2.ap())
            b1r_sb = constp.tile([1, f1], dt.bfloat16)
            nc.sync.dma_start(out=b1r_sb[:], in_=b1r.ap())
            b2r_sb = constp.tile([1, f2], dt.bfloat16)
            nc.sync.dma_start(out=b2r_sb[:], in_=b2r.ap())
            d1r_sb = constp.tile([1, nsh_pad], dt.bfloat16)
            nc.sync.dma_start(out=d1r_sb[:], in_=d1r.ap())
            ones_sb = constp.tile([1, P], dt.bfloat16)
            nc.vector.memset(ones_sb[:], 1.0)
            xo_sb = constp.tile([P, nb, f1], dt.bfloat16)
            nc.sync.dma_start(out=xo_sb[:],
                              in_=xo.ap().rearrange("(b p) f -> p b f", p=P))
            src2_sb = constp.tile([P, idx_cols], dt.int16)
            nc.scalar.dma_start(out=src2_sb[:], in_=src2.ap())

            qctr = [0]

            def layer(gather, src_sb, m_dram, s_dram, selftab, w_sb,
                      bias_lhsT, b_sb, fo, emit):
                tab = gather
                callno = [0]
                slot_base = 0
                for g in range(n_groups):
                    g0, g1 = g * G, min(g * G + G, nb)
                    gb = g1 - g0
                    half = gb * T2
                    s_t = smatp.tile([P, G * TT, P], dt.bfloat16, tag="smat")
                    # alternate HWDGE queues so the big S loads don't
                    # serialize behind each other on one sequencer
                    s_eng = nc.sync if g % 2 == 0 else nc.scalar
                    s_eng.dma_start(
                        out=s_t[:, : gb * TT, :],
                        in_=s_dram.ap()[:, g0 * TT * P : g1 * TT * P],
                    )
                    if gather:
                        msg = msgp.tile([P, G * TS, 2 * f1], dt.bfloat16,
                                        tag="msg")
                        for s0 in range(0, 2 * half, CSL):
                            s1_ = min(s0 + CSL, 2 * half)
                            i0 = (slot_base + s0) * P
                            n_idx = (s1_ - s0) * P
                            nc.gpsimd.dma_gather(
                                out_ap=msg[:, s0:s1_, :],
                                in_ap=_pairs_ap(tab, bounds[callno[0]], f1),
                                idxs_ap=src_sb[:, i0 // 16 : (i0 + n_idx) // 16],
                                num_idxs=n_idx,
                                num_idxs_reg=n_idx,
                                elem_size=2 * f1,
                                elem_step=2 * f1,
                                single_packet=False,
                                queue_num=qctr[0] % NQ,
                            )
                            qctr[0] += 1
                            callno[0] += 1
                    else:
                        msg = m1p.tile([P, G * TS, f1], dt.bfloat16, tag="m1")
                        m_eng = nc.scalar if g % 2 == 0 else nc.sync
                        m_eng.dma_start(
                            out=msg[:, : 2 * half, :],
                            in_=m_dram.ap()[
                                :, slot_base * f1 : (slot_base + 2 * half) * f1
                            ],
                        )
                    for j, b in enumerate(range(g0, g1)):
                        ps1 = ps1p.tile([f1, P], dt.float32, space="PSUM",
                                        tag="ps1")
                        for t in range(TT):
                            if t < TS:
                                parity, tsub = (0, t) if t < T2 else (1, t - T2)
                                if gather:
                                    # stream order: subtile-layer-major
                                    col = (tsub * 2 + parity) * gb + j
                                    lhsT = msg[:, col,
                                               parity * f1 : parity * f1 + f1]
                                else:
                                    lhsT = msg[:, parity * half + j * T2 + tsub,
                                               :]
                            else:
                                lhsT = selftab[:, b, :f1]
                            nc.tensor.matmul(
                                out=ps1[:],
                                lhsT=lhsT,
                                rhs=s_t[:, j * TT + t, :],
                                start=(t == 0),
                                stop=(t == TT - 1),
                            )
                        aggT = eplgp.tile([f1, P], dt.bfloat16, tag="aggT")
                        nc.scalar.copy(aggT[:], ps1[:])
                        ps2 = ps2p.tile([P, fo], dt.float32, space="PSUM",
                                        tag="ps2")
                        nc.tensor.matmul(out=ps2[:], lhsT=aggT[:], rhs=w_sb[:],
                                         start=True, stop=False)
                        nc.tensor.matmul(out=ps2[:], lhsT=bias_lhsT(b),
                                         rhs=b_sb[:], start=False, stop=True)
                        emit(b, ps2)
                    slot_base += gb * TS

            # ---- L1 ----
            r1s_sb = accp.tile([P, nb, f1], dt.bfloat16)
            r1s_own_r = r1s_own.ap().rearrange("(b p) f -> p b f", p=P)
            next_chunk = [0]

            def emit1(b, ps2):
                nc.scalar.activation(
                    out=r1s_sb[:, b, :], in_=ps2[:],
                    func=mybir.ActivationFunctionType.Relu,
                )
                k = next_chunk[0]
                if k < nch and b == cb[k + 1] - 1:
                    nc.sync.dma_start(out=r1s_own_r[:, cb[k] : cb[k + 1], :],
                                      in_=r1s_sb[:, cb[k] : cb[k + 1], :])
                    nc.gpsimd.collective_compute(
                        "AllGather",
                        mybir.AluOpType.bypass,
                        replica_groups=[list(range(N_CORES))],
                        ins=[r1s_own.ap()[cb[k] * P : cb[k + 1] * P, :].opt()],
                        outs=[r1s_full.ap()[off[k] : off[k + 1], :].opt()],
                    )
                    next_chunk[0] += 1

            layer(None, None, m1, s1, xo_sb, w1_sb,
                  lambda b: d1r_sb[0:1, b * P : (b + 1) * P], b1r_sb, f1, emit1)

            # ---- L2 ----
            out_sb = accp.tile([P, nb, f2], dt.float32)
            out_r = out.ap().rearrange("(b p) f -> p b f", p=P)
            out_chunk = [0]

            def emit2(b, ps2):
                nc.scalar.activation(
                    out=out_sb[:, b, :], in_=ps2[:],
                    func=mybir.ActivationFunctionType.Relu,
                )
                k = out_chunk[0]
                if k < nch and b == cb[k + 1] - 1:
                    nc.sync.dma_start(out=out_r[:, cb[k] : cb[k + 1], :],
                                      in_=out_sb[:, cb[k] : cb[k + 1], :])
                    out_chunk[0] += 1

            layer(r1s_full, src2_sb, None, s2, r1s_sb,
                  w2_sb, lambda b: ones_sb[0:1, :], b2r_sb, f2, emit2)

    nc.compile()
    return nc


_CACHE = {}


def kernel(x, edge_index, W1, b1, W2, b2, _want_profile=False):
    x = np.asarray(x)
    in_maps, cfg = _preprocess(x, edge_index, W1, b1, W2, b2)
    key = (cfg["n"], cfg["f1"], cfg["f2"], cfg["T2"], tuple(cfg["bounds"]))
    if key not in _CACHE:
        _CACHE[key] = _build(cfg)
    nc = _CACHE[key]
    node_ats = [m.pop("node_at") for m in in_maps]
    res = run_bass_kernel_spmd(
        nc, in_maps, core_ids=list(range(N_CORES)), trace=_want_profile
    )
    nsh = cfg["nsh"]
    full = np.empty((cfg["n"], cfg["f2"]), dtype=np.float32)
    for c in range(N_CORES):
        o = res.results[c]["out"]
        na = node_ats[c]
        occ = na >= 0
        full[c * nsh + na[occ]] = o[occ]
    if _want_profile:
        return full, res
    return full
